# revision 1
# baseline (speedup 1.0000x reference)
"""GNN message-passing (gather + segment-sum) Trainium2 kernel.

Strategy (dst-owner sharding, no collectives):
  - Core c owns output nodes [c*NPC, (c+1)*NPC).
  - Host counting-sorts edges into (core, bucket-group, src-block, bucket)
    sub-lists, pads each (bucket, src-block) sub-list to a multiple of 128
    edges (identical padded layout on every core -> one SPMD program).
  - Device per core:
      dma_gather x[src] rows (int16 block-local indices, 4 blocks of 25000
      rows) -> SBUF staging [128 edges, chunk, 64];
      one-hot of bucket-local dst via DVE is_equal against an iota row;
      PE matmul  psum[64 feats, 128 nodes] += msgs^T @ onehot  accumulated
      over a bucket's chunks; ACT copies psum -> SBUF out staging;
      one DMA of [64, NB*128] partial to HBM.
  - Host concatenates the 8 [64, 12500] shards -> [100000, 64].
"""

import sys

for _p in ("/opt/trn_rl_repo", "/root/.axon_site/_ro/trn_rl_repo"):
    if _p not in sys.path:
        sys.path.append(_p)

import numpy as np

from concourse import bass, mybir, tile, bacc
from concourse.bass_utils import run_bass_kernel_spmd

P = 128


def full_cfg():
    return dict(N=100000, D=64, E=1200000, CORES=8, BLOCK_ROWS=25000, GROUP=4,
                NSWQ=4, GMAX=7, STAG_BUFS=6, OH_BUFS=4, SRC_SORT=1, QUANT=64)


def make_layout(edge_index, cfg):
    """Counting-sort edges into the padded SPMD layout.

    Returns (Cmat, meta, per-core arrays).
    """
    N, CORES, BLOCK_ROWS, GROUP = cfg["N"], cfg["CORES"], cfg["BLOCK_ROWS"], cfg["GROUP"]
    NPC = N // CORES
    NB = -(-NPC // P)                       # buckets per core
    NBLK = -(-N // BLOCK_ROWS)              # src blocks
    NG = -(-NB // GROUP)                    # bucket groups

    src = np.asarray(edge_index[0], dtype=np.int64)
    dst = np.asarray(edge_index[1], dtype=np.int64)
    E = src.shape[0]

    core = dst // NPC
    dstl = dst - core * NPC
    bucket = dstl >> 7
    din = (dstl & 127).astype(np.float32)
    order = None
    if cfg.get("BALANCE"):
        # permute each core's buckets by size so slot k holds similar-sized
        # buckets on every core (shrinks the max-over-cores padding term)
        tot = np.bincount(core * NB + bucket, minlength=CORES * NB
                          ).reshape(CORES, NB)
        order = np.argsort(-tot, axis=1, kind="stable")      # slot -> bucket
        slot_of = np.empty_like(order)
        for c in range(CORES):
            slot_of[c, order[c]] = np.arange(NB)
        bucket = slot_of[core, bucket]                       # now slot index
    blk = src // BLOCK_ROWS
    srcl = (src - blk * BLOCK_ROWS).astype(np.int16)
    g = bucket // GROUP
    bing = bucket - g * GROUP

    # per-(core,bucket,blk) counts -> shared padded chunk counts
    cid = (core * NB + bucket) * NBLK + blk
    n = np.bincount(cid, minlength=CORES * NB * NBLK).reshape(CORES, NB, NBLK)
    Cmat = -(-n.max(axis=0) // P)           # [NB, NBLK] chunks
    Cmat[:, 0] = np.maximum(Cmat[:, 0], 1)  # every bucket gets >=1 chunk

    # sub-list start slots in layout order [g][blk][b in g]
    # sub-lists padded to multiples of QUANT (<=128); each (g,blk) unit
    # padded to a multiple of 128 so the chunk grid stays aligned.
    Q = int(cfg.get("QUANT", P))
    assert P % Q == 0
    S = np.zeros((NB, NBLK), dtype=np.int64)        # padded slots per sub-list
    nmax = n.max(axis=0)
    for b in range(NB):
        for bi in range(NBLK):
            m = int(nmax[b, bi])
            if bi == 0:
                m = max(m, 1)
            S[b, bi] = -(-m // Q) * Q
    sub_start = np.zeros((NB, NBLK), dtype=np.int64)
    units = {}                                       # (gi,bi) -> (t0, nck)
    pos = 0
    for gi in range(NG):
        bks = list(range(gi * GROUP, min((gi + 1) * GROUP, NB)))
        for bi in range(NBLK):
            u0 = pos
            for b in bks:
                sub_start[b, bi] = pos
                pos += int(S[b, bi])
            pos = -(-pos // P) * P                   # unit tail pad to x128
            units[(gi, bi)] = (u0 // P, (pos - u0) // P)
    T = pos // P                            # total chunks per core

    # slot -> owning bucket (-1 = pad-tail of a unit); includes sub-list pads
    owner = np.full(T * P, -1, dtype=np.int64)
    for b in range(NB):
        for bi in range(NBLK):
            owner[sub_start[b, bi]:sub_start[b, bi] + S[b, bi]] = b

    # per-unit run table: for each chunk, partition-runs of one bucket
    first_q = {}
    last_q = {}
    for b in range(NB):
        w = np.flatnonzero(owner == b)
        first_q[b], last_q[b] = int(w[0]), int(w[-1])
    runs = {}
    for (gi, bi), (t0, nck) in units.items():
        lst = []
        for tl in range(nck):
            base = (t0 + tl) * P
            rr = []
            j = 0
            while j < P:
                b = int(owner[base + j])
                k = j
                while k < P and int(owner[base + k]) == b:
                    k += Q
                if b >= 0:
                    st = first_q[b] >= base + j and first_q[b] < base + k
                    sp = last_q[b] >= base + j and last_q[b] < base + k
                    # decompose into PE-tile-aligned blocks (128/64/32)
                    blocks = []
                    jj = j
                    while jj < k:
                        for bs in (128, 64, 32):
                            if jj % bs == 0 and jj + bs <= k:
                                blocks.append((jj, bs))
                                jj += bs
                                break
                    for z, (bq, bl) in enumerate(blocks):
                        rr.append((b, bq, bl,
                                   st and z == 0,
                                   sp and z == len(blocks) - 1))
                j = k
            lst.append(rr)
        runs[(gi, bi)] = lst

    # per-edge slot assignment
    sort_key = ((core * NG + g) * NBLK + blk) * GROUP + bing
    if cfg.get("SRC_SORT"):
        perm = np.lexsort((src, sort_key))
    else:
        perm = np.argsort(sort_key, kind="stable")
    rid = sort_key[perm]
    starts = np.r_[0, np.flatnonzero(np.diff(rid)) + 1]
    counts = np.diff(np.r_[starts, E])
    rank = np.arange(E, dtype=np.int64) - np.repeat(starts, counts)
    slot = sub_start[bucket[perm], blk[perm]] + rank
    core_p = core[perm]

    if cfg.get("PADSKIP"):
        src_arr = np.full((CORES, T * P), -1, dtype=np.int16)
    else:
        src_arr = np.zeros((CORES, T * P), dtype=np.int16)
    if cfg.get("ZERO_SRC"):
        srcl[:] = 0
    dst_arr = np.full((CORES, T * P), -1.0, dtype=np.float32)
    src_arr[core_p, slot] = srcl[perm]
    dst_arr[core_p, slot] = din[perm]

    # per-(bucket,blk) call table with per-core valid counts
    calls = []          # (bucket, blk, t0_chunks, n_chunks)
    vcnt = None
    if cfg.get("PADSKIP"):
        for gi in range(NG):
            bks = range(gi * GROUP, min((gi + 1) * GROUP, NB))
            for bi in range(NBLK):
                for b in bks:
                    if Cmat[b, bi] > 0:
                        calls.append((b, bi, int(sub_start[b, bi]) // P, int(Cmat[b, bi])))
        vcnt = np.zeros((CORES, len(calls)), dtype=np.int32)
        for ci, (b, bi, t0, nchk) in enumerate(calls):
            vcnt[:, ci] = np.maximum(n[:, b, bi], 1)
        # calls with zero real edges on a core: make first pad valid (src 0)
        for ci, (b, bi, t0, nchk) in enumerate(calls):
            empty = n[:, b, bi] == 0
            if empty.any():
                src_arr[empty, t0 * P] = 0

    idx_np = np.empty((CORES, P, T * 8), dtype=np.int16)
    dstv_np = np.empty((CORES, P, T), dtype=np.float32)
    for c in range(CORES):
        w = src_arr[c].reshape(T * 8, 16).T          # [16, 8T]
        idx_np[c] = np.tile(w, (8, 1))
        dstv_np[c] = dst_arr[c].reshape(T, P).T      # [128, T]

    meta = dict(NPC=NPC, NB=NB, NBLK=NBLK, NG=NG, T=T, sub_start=sub_start,
                units=units, runs=runs, order=order,
                calls=calls if cfg.get("PADSKIP") else None)
    extras = {"vcnt": vcnt}
    meta["extras"] = extras
    return Cmat, meta, idx_np, dstv_np


def build_nc(Cmat, meta, cfg):
    N, D, CORES, BLOCK_ROWS, GROUP = (
        cfg["N"], cfg["D"], cfg["CORES"], cfg["BLOCK_ROWS"], cfg["GROUP"])
    NB, NBLK, NG, T = meta["NB"], meta["NBLK"], meta["NG"], meta["T"]
    sub_start = meta["sub_start"]
    f32 = mybir.dt.float32

    units, runs = meta["units"], meta["runs"]

    _gq = [0]
    nc = bacc.Bacc(
        None,
        target_bir_lowering=False,
        dynamic_dma_scratch_size=cfg.get("SCRATCH", 16384),
        num_swdge_queues=cfg.get("NSWQ", 1),
    )
    x = nc.dram_tensor("x", [N, D], f32, kind="ExternalInput")
    idx_in = nc.dram_tensor("idx", [P, T * 8], mybir.dt.int16, kind="ExternalInput")
    dstv_in = nc.dram_tensor("dstv", [P, T], f32, kind="ExternalInput")
    iota_in = nc.dram_tensor("iota", [P, P], f32, kind="ExternalInput")
    out = nc.dram_tensor("out", [D, NB * P], f32, kind="ExternalOutput")

    with tile.TileContext(nc) as tc:
        with (
            tc.tile_pool(name="persist", bufs=1) as persist,
            tc.tile_pool(name="stag", bufs=cfg.get("STAG_BUFS", 3)) as stagp,
            tc.tile_pool(name="oh", bufs=cfg.get("OH_BUFS", 2)) as ohp,
            tc.tile_pool(name="psum", bufs=8, space="PSUM") as psump,
        ):
            gsems = [nc.alloc_semaphore(f"gsem{q}") for q in range(cfg.get("NSWQ", 1))] if cfg.get("PREP") else None
            calls = meta.get("calls")
            PADSKIP = cfg.get("PADSKIP") and calls is not None
            if PADSKIP:
                ncalls = len(calls)
                CBMAX = max(c[3] for c in calls)
                vcnt_in = nc.dram_tensor("vcnt", [1, ncalls], mybir.dt.int32, kind="ExternalInput")
                vcnt_t = persist.tile([1, ncalls], mybir.dt.int32)
                nc.sync.dma_start(vcnt_t[:], vcnt_in[:])
                vregs = [nc.gpsimd.alloc_register(name=f"vr{i}") for i in range(4)]
                # map (bucket, blk) -> call index
                call_idx = {(b, bi): ci for ci, (b, bi, _, _) in enumerate(calls)}
                call_tiles = {}
            idx_t = persist.tile([P, T * 8], mybir.dt.int16)
            dstv_t = persist.tile([P, T], f32)
            iota_t = persist.tile([P, P], f32)
            outst = persist.tile([D, NB * P], f32)
            nc.sync.dma_start(idx_t[:], idx_in[:])
            nc.sync.dma_start(dstv_t[:], dstv_in[:])
            nc.sync.dma_start(iota_t[:], iota_in[:])

            if PADSKIP:
                NSLOT = cfg.get("STAG_BUFS", 3)
                stag_all = persist.tile([P, NSLOT, CBMAX, D], f32)
                nc.vector.memset(stag_all[:], 0.0)
            import contextlib
            reps = cfg.get("REPS", 0)
            loop_cm = tc.For_i(0, reps, 1) if reps else contextlib.nullcontext()
            with loop_cm:
              for gi in range(NG):
                  bks = list(range(gi * GROUP, min((gi + 1) * GROUP, NB)))
                  # one psum tile (= one bank) per bucket
                  ptiles = []
                  if not cfg.get("SKIP_COMPUTE"):
                    for h in range(len(bks)):
                      pt_tile = psump.tile([D, P], f32, tag="ps", name=f"ps_{gi}_{h}")
                      ptiles.append(pt_tile)

                  for bi in range(NBLK):
                      t0, nck = units[(gi, bi)]
                      if nck == 0:
                          continue
                      if PADSKIP:
                          NSWQ = cfg.get("NSWQ", 1)
                          # one gather call per (bucket, blk), runtime count reg
                          for b in bks:
                              C_b = int(Cmat[b, bi])
                              if C_b == 0:
                                  continue
                              ci = call_idx[(b, bi)]
                              if ci % 4 == 0:
                                  hi = min(ci + 4, ncalls)
                                  nc.gpsimd.reg_load(vregs[:hi - ci], vcnt_t[0:1, ci:hi])
                              tb = int(sub_start[b, bi]) // P
                              sg = stag_all[:, ci % NSLOT]
                              call_tiles[(b, bi)] = sg
                              q = _gq[0] % NSWQ
                              nc.gpsimd.dma_gather(
                                  sg[:, :C_b, :],
                                  x[bi * BLOCK_ROWS:(bi + 1) * BLOCK_ROWS, :],
                                  idx_t[:, tb * 8:(tb + C_b) * 8],
                                  C_b * P,
                                  vregs[ci % 4],
                                  D,
                                  queue_num=q,
                              )
                              _gq[0] += 1
                          stag = None
                      else:
                          stag = stagp.tile([P, nck, D], f32, tag="st")
                      GMAX = cfg.get("GMAX", 8)
                      NSWQ = cfg.get("NSWQ", 1)
                      if cfg.get("SKIP_GATHER"):
                          nc.gpsimd.memset(stag[:], 0.0)
                      if (not cfg.get("SKIP_GATHER")) and not PADSKIP:
                        for o in range(0, nck, GMAX):
                          w = min(GMAX, nck - o)
                          q = _gq[0] % NSWQ
                          nc.gpsimd.dma_gather(
                              stag[:, o:o + w, :],
                              x[bi * BLOCK_ROWS:(bi + 1) * BLOCK_ROWS, :],
                              idx_t[:, (t0 + o) * 8:(t0 + o + w) * 8],
                              w * P,
                              w * P,
                              D,
                              queue_num=q,
                              single_packet=cfg.get("SINGLE_PACKET", True),
                          )
                          _gq[0] += 1
                      if cfg.get("SKIP_COMPUTE"):
                          continue
                      oh = ohp.tile([P, nck, P], f32, tag="oh")
                      nc.vector.tensor_tensor(
                          out=oh[:],
                          in0=dstv_t[:, t0:t0 + nck].to_broadcast([P, nck, P]),
                          in1=iota_t[:, None, :].to_broadcast([P, nck, P]),
                          op=mybir.AluOpType.is_equal,
                      )
                      for tl in range(nck):
                          for (b, qpos, nq, st, sp) in runs[(gi, bi)][tl]:
                              pt = ptiles[b - bks[0]]
                              kw = {}
                              if qpos > 0:
                                  kw["tile_position"] = (qpos, 0)
                              nc.tensor.matmul(
                                  out=pt[:, :],
                                  lhsT=stag[qpos:qpos + nq, tl, :],
                                  rhs=oh[qpos:qpos + nq, tl, :],
                                  start=st,
                                  stop=sp,
                                  **kw,
                              )

                  for h, pt in enumerate(ptiles):
                      c0 = (bks[0] + h) * P
                      nc.scalar.copy(out=outst[:, c0:c0 + P], in_=pt[:, :])
                  if cfg.get("SKIP_COMPUTE") and gi == 0:
                      nc.vector.memset(outst[:], 0.0)
                  if cfg.get("OUTSPLIT", 1):
                      g0 = bks[0] * P
                      g1 = (bks[-1] + 1) * P
                      nc.sync.dma_start(out[:, g0:g1], outst[:, g0:g1])

            if not cfg.get("OUTSPLIT", 1):
                nc.sync.dma_start(out[:], outst[:])
    nc.finalize()
    return nc


_CACHE = {}


def _get_nc(Cmat, meta, cfg):
    key = (meta["sub_start"].tobytes(), meta["T"], cfg["N"], cfg["D"],
           cfg["CORES"], cfg.get("QUANT", P))
    if key not in _CACHE:
        _CACHE[key] = build_nc(Cmat, meta, cfg)
    return _CACHE[key]


def make_in_maps(x, idx_np, dstv_np, cfg, meta=None):
    CORES, D = cfg["CORES"], cfg["D"]
    xf = np.ascontiguousarray(np.asarray(x, dtype=np.float32))
    iota = np.broadcast_to(np.arange(P, dtype=np.float32), (P, P)).copy()
    maps = [
        {"x": xf, "idx": idx_np[c], "dstv": dstv_np[c], "iota": iota}
        for c in range(CORES)
    ]
    if meta is not None and meta.get("extras", {}).get("vcnt") is not None:
        vc = meta["extras"]["vcnt"]
        for c in range(CORES):
            maps[c]["vcnt"] = vc[c:c + 1]
    return maps


def assemble(shards, meta, cfg):
    N, D, CORES = cfg["N"], cfg["D"], cfg["CORES"]
    NPC, NB = meta["NPC"], meta["NB"]
    order = meta.get("order")
    if order is None:
        full = np.concatenate([sh[:, :NPC] for sh in shards], axis=1).T
        return np.ascontiguousarray(full)
    full = np.empty((N, D), dtype=np.float32)
    for c in range(CORES):
        for k in range(NB):
            gb = int(order[c][k])
            r0 = c * NPC + gb * P
            r1 = min(r0 + P, (c + 1) * NPC)
            if r0 >= r1:
                continue
            full[r0:r1] = shards[c][:, k * P:k * P + (r1 - r0)].T
    return full


def kernel(x, edge_index):
    cfg = full_cfg()
    Cmat, meta, idx_np, dstv_np = make_layout(edge_index, cfg)
    nc = _get_nc(Cmat, meta, cfg)
    in_maps = make_in_maps(x, idx_np, dstv_np, cfg, meta)
    res = run_bass_kernel_spmd(nc, in_maps, core_ids=list(range(cfg["CORES"])))
    shards = [res.results[c]["out"] for c in range(cfg["CORES"])]
    return assemble(shards, meta, cfg)

